# revision 28
# baseline (speedup 1.0000x reference)
"""Trainium2 Bass kernel for the 2-layer ConcatLSTM problem.

Sharding: data-parallel over batch (B=64 -> 8 per core), weights replicated.

Device design (per core, batch BC=8):
- Gate tile in PSUM: [128 partitions, 512 free]. Partition p = 32*j + b where
  j = H-slice (0..3) and b = batch (0..7); rows 32j+8..32j+31 are junk.
  Free f = gp*128 + h' where gp = gate position in order (i, f, o, g) and
  h = j*128 + h' is the hidden index.
- Recurrent matmul: h-stationary, 4 K-chunks x 4 col-groups (tile_position
  col tiling) streaming W_hh^T (permuted, bf16) as the moving operand.
  bf16 operands stream at 1 PE cycle/row (fp32 is 4); PSUM accumulates fp32.
- Wavefront schedule: layer-0 step w and layer-1 step w-LAG are emitted in
  the same iteration, so each layer's post-matmul ACT/DVE chain overlaps the
  other layer's PE matmul block. This keeps the PE continuously busy (ramped
  p-state) and roughly halves the serial per-step latency.
- Input projections xw = [x|ctx|1] @ W are computed in batched 128-row
  m-tiles (16 steps each) into SBUF rings (no DRAM round trip); a small
  SBUF->SBUF DMA per step rearranges one step's rows into the [32j+b, 512]
  gate layout for the DVE add.
- Layer-1 output h is staged in SBUF and written to DRAM once per 16 steps.
"""

import os
import sys

sys.path.insert(0, "/opt/trn_rl_repo")

import numpy as np
from contextlib import ExitStack

from concourse import bacc, tile, mybir
from concourse.bass_utils import run_bass_kernel_spmd
from concourse.masks import make_identity

T, B, I_IN, H, C = 512, 64, 256, 512, 64
G = 4 * H
NCORES = 8
BC = B // NCORES        # batch per core
K0 = 384                # 256 + 64 + 1 padded to 3*128
K1 = 640                # 512 + 64 + 1 padded to 5*128
LAG = 20                # layer-1 trails layer-0 by this many steps
F32 = mybir.dt.float32
BF16 = mybir.dt.bfloat16
AF = mybir.ActivationFunctionType

# device gate order (i, f, o, g); reference splits gates as (i, f, g, o)
ORIG_GATE = (0, 1, 3, 2)


def _gate_perm() -> np.ndarray:
    perm = np.empty(G, np.int64)
    for j in range(4):
        for gp, og in enumerate(ORIG_GATE):
            src = og * 512 + j * 128
            dst = j * 512 + gp * 128
            perm[dst:dst + 128] = np.arange(src, src + 128)
    return perm


PERM = _gate_perm()


def _load_bf16(tc, stg_pool, dst_pool, nc, name, dram, k_chunks, cols):
    """DMA fp32 [k*128, cols] from DRAM and convert to bf16 SBUF tiles."""
    out = []
    for k in range(k_chunks):
        s = stg_pool.tile([128, cols], F32, tag=f"{name}_s")
        nc.sync.dma_start(s[:], dram[k * 128:(k + 1) * 128, :])
        d = dst_pool.tile([128, cols], BF16, tag=f"{name}_r{k}")
        nc.vector.tensor_copy(d[:], s[:])
        out.append(d)
    return out


class _LayerState:
    """Per-layer recurrence state + emission helpers."""

    def __init__(self, nc, tc, ctx, name, wh_tiles, t_steps, ident,
                 store, n_slots, tp_pool, y_out=None):
        self.nc = nc
        self.name = name
        self.wh = wh_tiles
        self.t_steps = t_steps
        self.ident = ident
        self.store = store          # bf16 [128, n_slots*32] compact hT slots
        self.n_slots = n_slots
        self.plane = n_slots * 8
        self.store4 = store.rearrange("p (j f) -> p j f", j=4)
        self.y_out = y_out
        self.tp = tp_pool

        st = ctx.enter_context(tc.tile_pool(name=f"{name}_st", bufs=1))
        self.c_t = st.tile([128, 128], F32, tag=f"{name}_c")
        nc.gpsimd.memset(self.c_t[:], 0.0)
        self.zer = st.tile([128, 8], BF16, tag=f"{name}_z")
        nc.gpsimd.memset(self.zer[:], 0.0)

        self.xtp = ctx.enter_context(tc.tile_pool(name=f"{name}_xt", bufs=6))
        self.gp = ctx.enter_context(
            tc.tile_pool(name=f"{name}_g", bufs=2, space="PSUM"))
        self.sp = ctx.enter_context(tc.tile_pool(name=f"{name}_s", bufs=3))
        self.hp = ctx.enter_context(tc.tile_pool(name=f"{name}_h", bufs=3))
        if y_out is not None:
            self.yp = ctx.enter_context(tc.tile_pool(name=f"{name}_y", bufs=2))
            self.y_stage = None
        self.xw_tiles = {}          # m-tile index -> SBUF bf16 [128, G]

    def hT_write_dst(self, t):
        sl = t % self.n_slots
        return self.store4[:, :, sl * 8:(sl + 1) * 8]

    def hT_read(self, t, k):
        sl = t % self.n_slots
        off = k * self.plane + sl * 8
        return self.store[:, off:off + 8]

    def emit_dma(self, t):
        """xw_t rearrange: [8 rows, 4x512] m-tile -> [32j+b, 512] SBUF->SBUF."""
        nc = self.nc
        xt = self.xtp.tile([128, 512], BF16, tag=f"{self.name}_xtt")
        src_m = self.xw_tiles[t // 16]
        r = t % 16
        for j in range(4):
            nc.sync.dma_start(
                xt[32 * j:32 * j + BC, :],
                src_m[r * BC:(r + 1) * BC, j * 512:(j + 1) * 512])
        self.xt_cur = xt

    def emit_mm(self, t, k0, k1):
        """Recurrent matmul k-chunks [k0, k1) into the step's gate bank."""
        nc = self.nc
        if k0 == 0:
            self.ps_cur = self.gp.tile([128, 512], F32, tag=f"{self.name}_ps")
        ps = self.ps_cur
        for k in range(k0, k1):
            lh = self.zer[:] if t == 0 else self.hT_read(t - 1, k)
            for j in range(4):
                nc.tensor.matmul(
                    ps[32 * j:32 * j + BC, :],
                    lh,
                    self.wh[k][:, j * 512:(j + 1) * 512],
                    start=(k == 0),
                    stop=(k == 3),
                    tile_position=(0, 32 * j),
                )

    def emit_post(self, t):
        """gates += xw_t; activations; cell update. Leaves h in self.h_cur."""
        nc = self.nc
        ps = self.ps_cur
        nc.vector.tensor_add(ps[:], ps[:], self.xt_cur[:])

        # activations: sigmoid over (i, f, o), tanh over g
        s = self.sp.tile([128, 512], F32, tag=f"{self.name}_sa")
        nc.scalar.activation(s[:, 0:384], ps[:, 0:384], AF.Sigmoid)
        nc.scalar.activation(s[:, 384:512], ps[:, 384:512], AF.Tanh)

        # c = f*c + i*g ; h = o * tanh(c)
        ig = self.hp.tile([128, 128], F32, tag=f"{self.name}_ig")
        nc.vector.tensor_mul(ig[:], s[:, 0:128], s[:, 384:512])
        fc = self.hp.tile([128, 128], F32, tag=f"{self.name}_fc")
        nc.vector.tensor_mul(fc[:], s[:, 128:256], self.c_t[:])
        nc.vector.tensor_add(self.c_t[:], fc[:], ig[:])
        th = self.hp.tile([128, 128], F32, tag=f"{self.name}_th")
        nc.scalar.activation(th[:], self.c_t[:], AF.Tanh)
        h = self.hp.tile([128, 128], F32, tag=f"{self.name}_hh")
        nc.vector.tensor_mul(h[:], s[:, 256:384], th[:])
        self.h_cur = h

    def emit_tail(self, t):
        """Transpose h -> hT into the store slot, plus layer-1 y staging."""
        nc = self.nc
        h = self.h_cur
        last = (t == self.t_steps - 1)
        if not (last and self.y_out is not None):
            pt = self.tp.tile([128, 128], F32, tag="tp")
            nc.tensor.transpose(pt[:], h[:], self.ident[:])
            # per-plane copies so the next step's k=0 matmul only waits for
            # the first plane, overlapping the rest with its first MMs
            pt4 = pt.rearrange("p (j r) -> p j r", r=32)
            dst4 = self.hT_write_dst(t)
            for k in range(4):
                nc.vector.tensor_copy(dst4[:, k:k + 1, :], pt4[:, k:k + 1, 0:BC])

        # layer-1 output: stage h (fp32) and flush 16 steps per DMA set
        if self.y_out is not None:
            if t % 16 == 0:
                self.y_stage = self.yp.tile([128, 2048], F32,
                                            tag=f"{self.name}_ys")
            q = t % 16
            nc.vector.tensor_copy(
                self.y_stage[:, q * 128:(q + 1) * 128], h[:])
            if q == 15 or last:
                t0 = t - q
                for j in range(4):
                    # dst[b, i, e] = y[(t0+i)*8+b, j*128+e]; src partition dim
                    # (b) stays first on the SBUF side.
                    dst = self.y_out[t0 * BC:(t0 + q + 1) * BC, :] \
                        .rearrange("(i b) (j e) -> b i j e", b=BC, j=4) \
                        [:, :, j, :]
                    src = self.y_stage[32 * j:32 * j + BC, 0:(q + 1) * 128] \
                        .rearrange("b (i e) -> b i e", e=128)
                    nc.sync.dma_start(dst, src)


def build_nc(t_steps=T):
    assert t_steps % 16 == 0
    nc = bacc.Bacc("TRN2", target_bir_lowering=False, debug=False,
                   enable_asserts=False, num_devices=NCORES)
    mt = t_steps * BC // 128    # number of 16-step m-tiles

    xT = nc.dram_tensor("xT", [K0, t_steps * BC], F32, kind="ExternalInput").ap()
    w0 = nc.dram_tensor("w0", [K0, G], F32, kind="ExternalInput").ap()
    wh0 = nc.dram_tensor("wh0", [H, G], F32, kind="ExternalInput").ap()
    w1 = nc.dram_tensor("w1", [K1, G], F32, kind="ExternalInput").ap()
    wh1 = nc.dram_tensor("wh1", [H, G], F32, kind="ExternalInput").ap()
    aug4 = nc.dram_tensor("aug4", [128, 128], F32, kind="ExternalInput").ap()
    y = nc.dram_tensor("y", [t_steps * BC, H], F32, kind="ExternalOutput").ap()

    with tile.TileContext(nc) as tc:
        with ExitStack() as octx:
            misc = octx.enter_context(tc.tile_pool(name="misc", bufs=1))
            ident = misc.tile([128, 128], F32)
            make_identity(nc, ident[:])
            hist = misc.tile([128, t_steps * 32], BF16)
            ring1 = misc.tile([128, 64], BF16)

            wpool = octx.enter_context(tc.tile_pool(name="wts", bufs=1))
            with ExitStack() as sctx:
                stg = sctx.enter_context(tc.tile_pool(name="stg", bufs=2))
                w0t = _load_bf16(tc, stg, wpool, nc, "w0", w0, 3, G)
                wh0t = _load_bf16(tc, stg, wpool, nc, "wh0", wh0, 4, G)
                w1t = _load_bf16(tc, stg, wpool, nc, "w1", w1, 5, G)
                wh1t = _load_bf16(tc, stg, wpool, nc, "wh1", wh1, 4, G)
                aug4_stg = stg.tile([128, 128], F32, tag="aug4_s")
                nc.sync.dma_start(aug4_stg[:], aug4[:])
                aug4_sb = wpool.tile([128, 128], BF16, tag="aug4_r")
                nc.vector.tensor_copy(aug4_sb[:], aug4_stg[:])
                # x^T, converted to bf16 in 512-col chunks
                xts = []
                for k in range(3):
                    xk = wpool.tile([128, t_steps * BC], BF16, tag=f"xts{k}")
                    nchunk = t_steps * BC // 512
                    for ci in range(nchunk):
                        cs = stg.tile([128, 512], F32, tag="xstg")
                        nc.sync.dma_start(
                            cs[:],
                            xT[k * 128:(k + 1) * 128, ci * 512:(ci + 1) * 512])
                        nc.vector.tensor_copy(
                            xk[:, ci * 512:(ci + 1) * 512], cs[:])
                    xts.append(xk)

            # main pools
            xw0p = octx.enter_context(tc.tile_pool(name="xw0m", bufs=4))
            xw1p = octx.enter_context(tc.tile_pool(name="xw1m", bufs=4))
            pp = octx.enter_context(
                tc.tile_pool(name="xwps", bufs=2, space="PSUM"))
            tp = octx.enter_context(
                tc.tile_pool(name="tps", bufs=2, space="PSUM"))

            L0 = _LayerState(nc, tc, octx, "b", wh0t, t_steps, ident,
                             hist, t_steps, tp)
            L1 = _LayerState(nc, tc, octx, "d", wh1t, t_steps, ident,
                             ring1, 2, tp, y_out=y)

            plane = t_steps * 8

            from collections import deque
            quarters = deque()

            def emit_xw0_quarter(m, n):
                if n == 0:
                    xw0_new = xw0p.tile([128, G], BF16, tag="xw0t")
                    L0.xw_tiles[m] = xw0_new
                xm = L0.xw_tiles[m]
                psn = pp.tile([128, 512], F32, tag="xwpsn")
                for k in range(3):
                    nc.tensor.matmul(
                        psn[:],
                        xts[k][:, m * 128:(m + 1) * 128],
                        w0t[k][:, n * 512:(n + 1) * 512],
                        start=(k == 0), stop=(k == 2))
                nc.scalar.copy(xm[:, n * 512:(n + 1) * 512], psn[:])

            def emit_xw1_quarter(m, n):
                if n == 0:
                    xw1_new = xw1p.tile([128, G], BF16, tag="xw1t")
                    L1.xw_tiles[m] = xw1_new
                xm = L1.xw_tiles[m]
                psn = pp.tile([128, 512], F32, tag="xwpsn")
                for k in range(5):
                    if k == 4:
                        lhs = aug4_sb[:]
                    else:
                        lhs = hist[:, k * plane + m * 128:
                                   k * plane + (m + 1) * 128]
                    nc.tensor.matmul(
                        psn[:],
                        lhs,
                        w1t[k][:, n * 512:(n + 1) * 512],
                        start=(k == 0), stop=(k == 4))
                nc.scalar.copy(xm[:, n * 512:(n + 1) * 512], psn[:])

            # prologue: first two xw0 m-tiles in full
            for m in range(min(2, mt)):
                for n in range(4):
                    emit_xw0_quarter(m, n)

            # Wavefront main loop: A = L0 step w, B = L1 step w-LAG.
            # Each step's transpose+hT-copy (tail) is tucked into the MIDDLE
            # of the other layer's matmul block, and the copy is emitted
            # before the next DVE add, so the serial cycle
            #   MM(X) -> chain(X) -> transpose -> copy -> MM(X+1)
            # of one layer overlaps the other layer's MM block with no PE or
            # DVE queuing stalls.
            for w in range(t_steps + LAG + 1):
                a = w
                b = w - LAG
                pb = w - 1 - LAG
                if a < t_steps:
                    L0.emit_dma(a)
                    L0.emit_mm(a, 0, 2)
                if 0 <= pb < t_steps:
                    L1.emit_tail(pb)
                if a < t_steps:
                    L0.emit_mm(a, 2, 4)
                    L0.emit_post(a)
                if 0 <= b < t_steps:
                    L1.emit_dma(b)
                    L1.emit_mm(b, 0, 3)
                if a < t_steps:
                    L0.emit_tail(a)
                if 0 <= b < t_steps:
                    L1.emit_mm(b, 3, 4)
                    L1.emit_post(b)
                if w % 16 == 0:
                    m1 = w // 16 - 1
                    if 0 <= m1 < mt:
                        for n in range(4):
                            quarters.append((emit_xw1_quarter, m1, n))
                    m0 = w // 16 + 2
                    if m0 < mt:
                        for n in range(4):
                            quarters.append((emit_xw0_quarter, m0, n))
                # drain up to two phase quarters per wavefront into PE slack
                for _ in range(2):
                    if quarters:
                        fn, m, n = quarters.popleft()
                        fn(m, n)

    nc.finalize()
    return nc


def host_inputs(x, date_contexts, w_ih0, w_hh0, w_mh0, b0,
                w_ih1, w_hh1, w_mh1, b1, t_steps=T):
    """Build per-core input maps (layout prep only, no heavy math)."""
    f = lambda a: np.ascontiguousarray(np.asarray(a, np.float32))
    x, ctx = f(x), f(date_contexts)
    w0aug = np.zeros((K0, G), np.float32)
    w0aug[0:I_IN] = f(w_ih0).T
    w0aug[I_IN:I_IN + C] = f(w_mh0).T
    w0aug[I_IN + C] = f(b0)
    w0aug = np.ascontiguousarray(w0aug[:, PERM])
    wh0p = np.ascontiguousarray(f(w_hh0).T[:, PERM])
    w1aug = np.zeros((K1, G), np.float32)
    w1aug[0:H] = f(w_ih1).T
    w1aug[H:H + C] = f(w_mh1).T
    w1aug[H + C] = f(b1)
    w1aug = np.ascontiguousarray(w1aug[:, PERM])
    wh1p = np.ascontiguousarray(f(w_hh1).T[:, PERM])

    in_maps = []
    for c in range(NCORES):
        bs = slice(c * BC, (c + 1) * BC)
        xTc = np.zeros((K0, t_steps, BC), np.float32)
        xTc[0:I_IN] = np.moveaxis(x[:t_steps, bs, :], 2, 0)
        xTc[I_IN:I_IN + C] = ctx[bs].T[:, None, :]
        xTc[I_IN + C] = 1.0
        a4 = np.zeros((128, 128), np.float32)
        a4[0:C] = np.broadcast_to(
            ctx[bs].T[:, None, :], (C, 16, BC)).reshape(C, 128)
        a4[C] = 1.0
        in_maps.append({
            "xT": np.ascontiguousarray(xTc.reshape(K0, t_steps * BC)),
            "w0": w0aug, "wh0": wh0p, "w1": w1aug, "wh1": wh1p, "aug4": a4,
        })
    return in_maps


_NC_CACHE = {}


def _get_nc(t_steps=T):
    if t_steps not in _NC_CACHE:
        _NC_CACHE[t_steps] = build_nc(t_steps)
    return _NC_CACHE[t_steps]


def kernel(x, date_contexts, w_ih0, w_hh0, w_mh0, b0,
           w_ih1, w_hh1, w_mh1, b1):
    t_steps = int(os.environ.get("LSTM_T_STEPS", T))
    in_maps = host_inputs(x, date_contexts, w_ih0, w_hh0, w_mh0, b0,
                          w_ih1, w_hh1, w_mh1, b1, t_steps)
    nc = _get_nc(t_steps)
    res = run_bass_kernel_spmd(nc, in_maps, core_ids=list(range(NCORES)))
    out = np.stack(
        [res.results[c]["y"].reshape(t_steps, BC, H) for c in range(NCORES)],
        axis=1,
    )  # [T, NCORES, BC, H]
    return np.ascontiguousarray(
        out.reshape(t_steps, B, H)).astype(np.float32)
